# revision 11
# baseline (speedup 1.0000x reference)
"""Self-contained Trainium2 Bass kernel for nn_MultiHeadAttention_75273596829862.

Sharding: 8 cores = 2 batches x 4 head-groups (3 heads each). Each core
computes QKV projections for its heads, transposed-softmax attention, and a
partial output projection out_pT[768,2048]; the host sums 4 partials per
batch and adds bo.

Layout notes:
- Host pre-transposes X -> X^T [768, 2048] and casts to bf16.
- wcat columns: [Q_h0|Q_h1 | K_h0|K_h1 | Q_h2|K_h2 | V_h0|V_h1 | V_h2|pad]
  so projection m-tiles keep per-head Q/K slices on matching partition
  halves (h1 ops run at base partition 64 via tile_position).
- scores are computed transposed (t on partitions) so softmax needs no
  P-transpose; the V~ ones-column makes the PV matmul emit the softmax
  denominator as row 64 for free.
"""

import sys

for p in ("/opt/trn_rl_repo", "/root/.axon_site/_ro/trn_rl_repo"):
    if p not in sys.path:
        sys.path.insert(0, p)

import numpy as np
import ml_dtypes

BF16 = ml_dtypes.bfloat16

# Problem constants (hardcoded per spec)
B, S, DM = 2, 2048, 768
H, HD = 12, 64
NCORES = 8
HPC = 3              # heads per core
WCOLS = 640          # packed projection columns (576 used + 64 pad)
KT = DM // 128       # 6 k-tiles of the contraction dim
ST = S // 128        # 16 t-tiles of the sequence
CH = 512             # matmul moving free-dim chunk (one PSUM bank)
NCH = S // CH        # 4 chunks of the s dimension

_compiled = None


def _build(loop_reps=0, dbg=False):
    import concourse.bacc as bacc
    import concourse.tile as tile
    import concourse.mybir as mybir
    from concourse.bass import ts, ds
    from concourse.masks import make_identity

    dt = mybir.dt
    AF = mybir.ActivationFunctionType

    nc = bacc.Bacc("TRN2", target_bir_lowering=False, debug=False)

    xqT = nc.dram_tensor("xqT", [DM, S], dt.bfloat16, kind="ExternalInput").ap()
    xkT = nc.dram_tensor("xkT", [DM, S], dt.bfloat16, kind="ExternalInput").ap()
    xvT = nc.dram_tensor("xvT", [DM, S], dt.bfloat16, kind="ExternalInput").ap()
    wcat = nc.dram_tensor("wcat", [DM, WCOLS], dt.bfloat16,
                          kind="ExternalInput").ap()
    wo = nc.dram_tensor("wo", [HPC * HD, DM], dt.bfloat16,
                        kind="ExternalInput").ap()
    bcat = nc.dram_tensor("bcat", [WCOLS], dt.float32, kind="ExternalInput").ap()
    bv = nc.dram_tensor("bv", [HPC * HD], dt.float32, kind="ExternalInput").ap()
    out_pT = nc.dram_tensor("out_pT", [DM, S], dt.bfloat16,
                            kind="ExternalOutput").ap()

    with tile.TileContext(nc) as tc:
        def body():
          with tc.tile_pool(name="consts", bufs=1) as cpool:
            # ---- coalesced input DMAs (one per DRAM tensor) ----
            xq_sb = cpool.tile([128, KT * S], dt.bfloat16, tag="xq")
            nc.sync.dma_start(
                out=xq_sb.rearrange("p (k s) -> p k s", k=KT),
                in_=xqT.rearrange("(k p) s -> p k s", p=128))
            xk_sb = cpool.tile([128, KT * S], dt.bfloat16, tag="xk")
            nc.sync.dma_start(
                out=xk_sb.rearrange("p (k s) -> p k s", k=KT),
                in_=xkT.rearrange("(k p) s -> p k s", p=128))
            xv_sb = cpool.tile([128, KT * S], dt.bfloat16, tag="xv")
            nc.sync.dma_start(
                out=xv_sb.rearrange("p (k s) -> p k s", k=KT),
                in_=xvT.rearrange("(k p) s -> p k s", p=128))
            wc_sb = cpool.tile([128, KT * WCOLS], dt.bfloat16, tag="wc")
            nc.sync.dma_start(
                out=wc_sb.rearrange("p (k c) -> p k c", k=KT),
                in_=wcat.rearrange("(k p) c -> p k c", p=128))
            wo_sb = cpool.tile([HD, HPC * DM], dt.bfloat16, tag="wos")
            nc.sync.dma_start(
                out=wo_sb.rearrange("d (h e) -> d h e", h=HPC),
                in_=wo.rearrange("(h d) e -> d h e", d=HD))
            bqk_sb = cpool.tile([128, WCOLS // 128], dt.float32, tag="bqk")
            nc.sync.dma_start(out=bqk_sb,
                              in_=bcat.rearrange("(m p) -> p m", p=128))
            bv_sb = cpool.tile([HD, HPC], dt.float32, tag="bvs")
            nc.sync.dma_start(out=bv_sb,
                              in_=bv.rearrange("(h d) -> d h", d=HD))
            ident = cpool.tile([128, 128], dt.bfloat16, tag="ident")
            make_identity(nc, ident)

            def xq_k(k):
                return xq_sb[:, ts(k, S)]

            def xk_k(k):
                return xk_sb[:, ts(k, S)]

            def xv_k(k):
                return xv_sb[:, ts(k, S)]

            def wc_k(k, c0, w):
                return wc_sb[:, k * WCOLS + c0: k * WCOLS + c0 + w]

            # projection outputs (d' on partitions)
            qk01_sb = cpool.tile([128, S], dt.bfloat16, tag="qk01")
            k01_sb = cpool.tile([128, S], dt.bfloat16, tag="k01")
            qk2_sb = cpool.tile([128, S], dt.bfloat16, tag="qk2")
            kT2_sb = cpool.tile([HD, S], dt.bfloat16, tag="kT2")
            v01_sb = cpool.tile([128, S], dt.bfloat16, tag="v01")
            v2_sb = cpool.tile([HD, S], dt.bfloat16, tag="v2")
            # V~ tiles: vv01[t] [128, 130] = [V_h0|1|V_h1|1]; vv2[t] [128, 65]
            vv01 = [cpool.tile([128, 2 * (HD + 1)], dt.bfloat16,
                               tag=f"vv01_{t}", name=f"vv01_{t}")
                    for t in range(ST)]
            vv2 = [cpool.tile([128, HD + 1], dt.bfloat16,
                              tag=f"vv2_{t}", name=f"vv2_{t}")
                   for t in range(ST)]
            onrm_sb = [cpool.tile([HD, S], dt.bfloat16, tag=f"onrm{h}",
                                  name=f"onrm{h}")
                       for h in range(HPC)]

            # ---------------- projections ----------------
            # groups: (wcat col0, M, xsrc, psum row0, psum tag, bias col)
            groups = [
                (0, 128, xq_k, 0, 0, 0),     # Q_h0|Q_h1
                (128, 128, xk_k, 0, 1, 1),   # K_h0|K_h1
                (256, 64, xq_k, 0, 2, 2),    # Q_h2
                (320, 64, xk_k, 64, 2, 2),   # K_h2 (upper rows)
                (384, 128, xv_k, 0, 3, 3),   # V_h0|V_h1
                (512, 64, xv_k, 0, 4, 4),    # V_h2
            ]
            dests = {0: qk01_sb, 1: k01_sb, 2: qk2_sb, 3: v01_sb, 4: v2_sb}
            with tc.tile_pool(name="pproj", bufs=1, space="PSUM") as pqk:
                for c in range(NCH):
                    pts = {}
                    for col0, m, xf, row0, pi, bi in groups:
                        if pi not in pts:
                            pts[pi] = pqk.tile([128, CH], dt.float32,
                                               tag=f"pp{pi}", name=f"pp{pi}")
                        pt = pts[pi]
                        for k in range(KT):
                            nc.tensor.matmul(
                                pt[row0:row0 + m, :],
                                wc_k(k, col0, m),
                                xf(k)[:, ts(c, CH)],
                                start=(k == 0),
                                stop=(k == KT - 1),
                            )
                    for pi, dst in dests.items():
                        rows = dst.shape[0]
                        nc.vector.tensor_scalar_add(
                            dst[:, ts(c, CH)],
                            pts[pi][0:rows, :],
                            bqk_sb[0:rows, pi:pi + 1],
                        )

            # K_h2 lives at partitions 64:128 of qk2_sb; realign to base 0
            nc.sync.dma_start(out=kT2_sb, in_=qk2_sb[64:128, :])

            # ---- V transposes: [d', t-slice] -> [t, d'] + ones columns ----
            with tc.tile_pool(name="ptr", bufs=4, space="PSUM") as ptr:
                for t in range(ST):
                    tr1 = ptr.tile([128, 128], dt.bfloat16, tag="tr1",
                                   name="tr1")
                    nc.tensor.transpose(tr1, v01_sb[:, ts(t, 128)], ident)
                    nc.vector.tensor_copy(
                        vv01[t].rearrange("p (h x) -> p h x", h=2)[:, :, 0:HD],
                        tr1.rearrange("p (h x) -> p h x", h=2),
                    )
                    nc.gpsimd.memset(
                        vv01[t].rearrange("p (h x) -> p h x", h=2)[:, :,
                                                                   HD:HD + 1],
                        1.0)
                    tr2 = ptr.tile([128, HD], dt.bfloat16, tag="tr2",
                                   name="tr2")
                    nc.tensor.transpose(tr2, v2_sb[:, ts(t, 128)],
                                        ident[0:HD, 0:HD])
                    nc.vector.tensor_copy(vv2[t][:, 0:HD], tr2)
                    nc.gpsimd.memset(vv2[t][:, HD:HD + 1], 1.0)

            qk_heads = [
                (qk01_sb[0:HD, :], k01_sb[0:HD, :]),
                (qk01_sb[HD:128, :], k01_sb[HD:128, :]),
                (qk2_sb[0:HD, :], kT2_sb),
            ]
            vv_heads = [
                lambda t: vv01[t][:, 0:HD + 1],
                lambda t: vv01[t][:, HD + 1:2 * (HD + 1)],
                lambda t: vv2[t],
            ]

            # ---------------- attention (pipelined per t-tile) --------------
            with (
                tc.tile_pool(name="ps", bufs=2, space="PSUM") as psp,
                tc.tile_pool(name="po", bufs=1, space="PSUM") as pop,
                tc.tile_pool(name="exps", bufs=4) as epool,
                tc.tile_pool(name="smalls", bufs=4) as spool,
            ):
                for h in range(HPC):
                    qT_h, kT_h = qk_heads[h]
                    vv_h = vv_heads[h]
                    po = [pop.tile([HD + 1, CH], dt.float32, tag=f"po{c}",
                                   name=f"po{c}")
                          for c in range(NCH)]
                    for t in range(ST):
                        for half in range(2):
                            ps = psp.tile([128, 2 * CH], dt.float32, tag="ps",
                                          name="ps")
                            for j in range(2):
                                c = 2 * half + j
                                nc.tensor.matmul(
                                    ps[:, ts(j, CH)],
                                    kT_h[:, ts(t, 128)],
                                    qT_h[:, ts(c, CH)],
                                )
                            et = epool.tile([128, 2 * CH], dt.bfloat16,
                                            tag="exp", name="et")
                            nc.scalar.activation(et, ps, AF.Exp, scale=0.125)
                            for j in range(2):
                                c = 2 * half + j
                                nc.tensor.matmul(
                                    po[c],
                                    vv_h(t),
                                    et[:, ts(j, CH)],
                                    start=(t == 0),
                                    stop=(t == ST - 1),
                                )
                    # normalization per chunk
                    for c in range(NCH):
                        dtile = spool.tile([1, CH], dt.float32, tag="den",
                                           name="dtile")
                        nc.vector.tensor_copy(dtile, po[c][HD:HD + 1, :])
                        rtile = spool.tile([1, CH], dt.float32, tag="rec",
                                           name="rtile")
                        nc.vector.reciprocal_approx_fast(out=rtile, in_=dtile)
                        bcst = spool.tile([HD, CH], dt.float32, tag="bcast",
                                          name="bcst")
                        nc.gpsimd.partition_broadcast(bcst, rtile)
                        dst = onrm_sb[h][:, ts(c, CH)]
                        nc.vector.tensor_mul(dst, po[c][0:HD, :], bcst)
                        nc.vector.tensor_scalar_add(dst, dst,
                                                    bv_sb[:, h:h + 1])

            # ---------------- output projection ----------------
            with (
                tc.tile_pool(name="pout", bufs=2, space="PSUM") as poutp,
                tc.tile_pool(name="outs", bufs=2) as opool,
            ):
                for e in range(KT):
                    ot = opool.tile([128, S], dt.bfloat16, tag="ot", name="ot")
                    for c in range(NCH):
                        pout = poutp.tile([128, CH], dt.float32, tag="pout",
                                          name="pout")
                        for h in range(HPC):
                            nc.tensor.matmul(
                                pout,
                                wo_sb[:, h * DM + e * 128: h * DM + (e + 1) * 128],
                                onrm_sb[h][:, ts(c, CH)],
                                start=(h == 0),
                                stop=(h == HPC - 1),
                            )
                        nc.vector.tensor_copy(ot[:, ts(c, CH)], pout)
                    nc.gpsimd.dma_start(out=out_pT[ts(e, 128), :], in_=ot)

        if loop_reps > 1:
            with tc.For_i(0, loop_reps, 1):
                body()
        else:
            body()

    nc.compile()
    return nc


def _shard_inputs(query, key, value, wq, bq, wk, bk, wv, bv, wo, bo):
    """Build the 8 per-core input maps."""
    f32 = np.float32
    in_maps = []
    for core in range(NCORES):
        b = core // 4
        h0 = (core % 4) * HPC
        cs = slice(h0 * HD, (h0 + HPC) * HD)
        wq_s = np.asarray(wq[:, cs], f32)
        wk_s = np.asarray(wk[:, cs], f32)
        wv_s = np.asarray(wv[:, cs], f32)
        pad = np.zeros((DM, HD), f32)
        wcat = np.concatenate(
            [wq_s[:, 0:128], wk_s[:, 0:128], wq_s[:, 128:192],
             wk_s[:, 128:192], wv_s[:, 0:128], wv_s[:, 128:192], pad], axis=1)
        bq_s, bk_s, bv_s = (np.asarray(x[cs], f32) for x in (bq, bk, bv))
        bcat = np.concatenate([bq_s[0:128], bk_s[0:128], bq_s[128:192],
                               bk_s[128:192], bv_s[0:128], bv_s[128:192],
                               np.zeros(64, f32)])
        in_maps.append({
            "xqT": np.ascontiguousarray(np.asarray(query, f32)[b].T).astype(BF16),
            "xkT": np.ascontiguousarray(np.asarray(key, f32)[b].T).astype(BF16),
            "xvT": np.ascontiguousarray(np.asarray(value, f32)[b].T).astype(BF16),
            "wcat": np.ascontiguousarray(wcat).astype(BF16),
            "wo": np.ascontiguousarray(np.asarray(wo, f32)[cs, :]).astype(BF16),
            "bcat": np.ascontiguousarray(bcat),
            "bv": np.ascontiguousarray(bv_s),
        })
    return in_maps


def kernel(query, key, value, wq, bq, wk, bk, wv, bv, wo, bo):
    global _compiled
    from concourse.bass_utils import run_bass_kernel_spmd

    if _compiled is None:
        _compiled = _build()
    nc = _compiled

    in_maps = _shard_inputs(query, key, value, wq, bq, wk, bk, wv, bv, wo, bo)
    res = run_bass_kernel_spmd(nc, in_maps, list(range(NCORES)))

    out = np.zeros((B, S, DM), dtype=np.float32)
    for core in range(NCORES):
        b = core // 4
        out[b] += res.results[core]["out_pT"].astype(np.float32).T
    out += np.asarray(bo, dtype=np.float32)[None, None, :]
    return out


# revision 12
# speedup vs baseline: 6.4605x; 6.4605x over previous
"""Self-contained Trainium2 Bass kernel for nn_MultiHeadAttention_75273596829862.

Sharding: 8 cores = 2 batches x 4 head-groups (3 heads each). Each core
computes QKV projections for its heads, transposed-softmax attention, and a
partial output projection out_pT[768,2048]; the host sums 4 partials per
batch and adds bo.

Layout notes:
- Host pre-transposes X -> X^T [768, 2048] and casts to bf16.
- wcat columns: [Q_h0|Q_h1 | K_h0|K_h1 | Q_h2|K_h2 | V_h0|V_h1 | V_h2|pad]
  so projection m-tiles keep per-head Q/K slices on matching partition
  halves (h1 ops run at base partition 64 via tile_position).
- scores are computed transposed (t on partitions) so softmax needs no
  P-transpose; the V~ ones-column makes the PV matmul emit the softmax
  denominator as row 64 for free.
"""

import sys

for p in ("/opt/trn_rl_repo", "/root/.axon_site/_ro/trn_rl_repo"):
    if p not in sys.path:
        sys.path.insert(0, p)

import numpy as np
import ml_dtypes

BF16 = ml_dtypes.bfloat16

# Problem constants (hardcoded per spec)
B, S, DM = 2, 2048, 768
H, HD = 12, 64
NCORES = 8
HPC = 3              # heads per core
WCOLS = 640          # packed projection columns (576 used + 64 pad)
KT = DM // 128       # 6 k-tiles of the contraction dim
ST = S // 128        # 16 t-tiles of the sequence
CH = 512             # matmul moving free-dim chunk (one PSUM bank)
NCH = S // CH        # 4 chunks of the s dimension

_compiled = None


def _build(loop_reps=0, dbg=False):
    import concourse.bacc as bacc
    import concourse.tile as tile
    import concourse.mybir as mybir
    from concourse.bass import ts, ds
    from concourse.masks import make_identity

    dt = mybir.dt
    AF = mybir.ActivationFunctionType

    nc = bacc.Bacc("TRN2", target_bir_lowering=False, debug=False)

    xqT = nc.dram_tensor("xqT", [DM, S], dt.bfloat16, kind="ExternalInput").ap()
    xkT = nc.dram_tensor("xkT", [DM, S], dt.bfloat16, kind="ExternalInput").ap()
    xvT = nc.dram_tensor("xvT", [DM, S], dt.bfloat16, kind="ExternalInput").ap()
    wcat = nc.dram_tensor("wcat", [DM, WCOLS], dt.bfloat16,
                          kind="ExternalInput").ap()
    wo = nc.dram_tensor("wo", [HPC * HD, DM], dt.bfloat16,
                        kind="ExternalInput").ap()
    bcat = nc.dram_tensor("bcat", [WCOLS], dt.float32, kind="ExternalInput").ap()
    bv = nc.dram_tensor("bv", [HPC * HD], dt.float32, kind="ExternalInput").ap()
    out_pT = nc.dram_tensor("out_pT", [DM, S], dt.bfloat16,
                            kind="ExternalOutput").ap()

    with tile.TileContext(nc) as tc:
        def body():
          with tc.tile_pool(name="consts", bufs=1) as cpool:
            # ---- coalesced input DMAs (one per DRAM tensor) ----
            xq_sb = cpool.tile([128, KT * S], dt.bfloat16, tag="xq")
            nc.sync.dma_start(
                out=xq_sb.rearrange("p (k s) -> p k s", k=KT),
                in_=xqT.rearrange("(k p) s -> p k s", p=128))
            xk_sb = cpool.tile([128, KT * S], dt.bfloat16, tag="xk")
            nc.sync.dma_start(
                out=xk_sb.rearrange("p (k s) -> p k s", k=KT),
                in_=xkT.rearrange("(k p) s -> p k s", p=128))
            xv_sb = cpool.tile([128, KT * S], dt.bfloat16, tag="xv")
            nc.sync.dma_start(
                out=xv_sb.rearrange("p (k s) -> p k s", k=KT),
                in_=xvT.rearrange("(k p) s -> p k s", p=128))
            wc_sb = cpool.tile([128, KT * WCOLS], dt.bfloat16, tag="wc")
            nc.sync.dma_start(
                out=wc_sb.rearrange("p (k c) -> p k c", k=KT),
                in_=wcat.rearrange("(k p) c -> p k c", p=128))
            wo_sb = cpool.tile([HD, HPC * DM], dt.bfloat16, tag="wos")
            nc.sync.dma_start(
                out=wo_sb.rearrange("d (h e) -> d h e", h=HPC),
                in_=wo.rearrange("(h d) e -> d h e", d=HD))
            bqk_sb = cpool.tile([128, WCOLS // 128], dt.float32, tag="bqk")
            nc.sync.dma_start(out=bqk_sb,
                              in_=bcat.rearrange("(m p) -> p m", p=128))
            bv_sb = cpool.tile([HD, HPC], dt.float32, tag="bvs")
            nc.sync.dma_start(out=bv_sb,
                              in_=bv.rearrange("(h d) -> d h", d=HD))
            ident = cpool.tile([128, 128], dt.bfloat16, tag="ident")
            make_identity(nc, ident)

            def xq_k(k):
                return xq_sb[:, ts(k, S)]

            def xk_k(k):
                return xk_sb[:, ts(k, S)]

            def xv_k(k):
                return xv_sb[:, ts(k, S)]

            def wc_k(k, c0, w):
                return wc_sb[:, k * WCOLS + c0: k * WCOLS + c0 + w]

            # projection outputs (d' on partitions)
            qk01_sb = cpool.tile([128, S], dt.bfloat16, tag="qk01")
            k01_sb = cpool.tile([128, S], dt.bfloat16, tag="k01")
            qk2_sb = cpool.tile([128, S], dt.bfloat16, tag="qk2")
            kT2_sb = cpool.tile([HD, S], dt.bfloat16, tag="kT2")
            v01_sb = cpool.tile([128, S], dt.bfloat16, tag="v01")
            v2_sb = cpool.tile([HD, S], dt.bfloat16, tag="v2")
            # V~ tiles: vv01[t] [128, 130] = [V_h0|1|V_h1|1]; vv2[t] [128, 65]
            vv01 = [cpool.tile([128, 2 * (HD + 1)], dt.bfloat16,
                               tag=f"vv01_{t}", name=f"vv01_{t}")
                    for t in range(ST)]
            vv2 = [cpool.tile([128, HD + 1], dt.bfloat16,
                              tag=f"vv2_{t}", name=f"vv2_{t}")
                   for t in range(ST)]
            onrm_sb = [cpool.tile([HD, S], dt.bfloat16, tag=f"onrm{h}",
                                  name=f"onrm{h}")
                       for h in range(HPC)]

            # ---------------- projections ----------------
            # groups: (wcat col0, M, xsrc, psum row0, psum tag, bias col)
            groups = [
                (0, 128, xq_k, 0, 0, 0),     # Q_h0|Q_h1
                (128, 128, xk_k, 0, 1, 1),   # K_h0|K_h1
                (256, 64, xq_k, 0, 2, 2),    # Q_h2
                (320, 64, xk_k, 64, 2, 2),   # K_h2 (upper rows)
                (384, 128, xv_k, 0, 3, 3),   # V_h0|V_h1
                (512, 64, xv_k, 0, 4, 4),    # V_h2
            ]
            dests = {0: qk01_sb, 1: k01_sb, 2: qk2_sb, 3: v01_sb, 4: v2_sb}
            with tc.tile_pool(name="pproj", bufs=1, space="PSUM") as pqk:
                for c in range(NCH):
                    pts = {}
                    for col0, m, xf, row0, pi, bi in groups:
                        if pi not in pts:
                            pts[pi] = pqk.tile([128, CH], dt.float32,
                                               tag=f"pp{pi}", name=f"pp{pi}")
                        pt = pts[pi]
                        for k in range(KT):
                            nc.tensor.matmul(
                                pt[row0:row0 + m, :],
                                wc_k(k, col0, m),
                                xf(k)[:, ts(c, CH)],
                                start=(k == 0),
                                stop=(k == KT - 1),
                            )
                    for pi, dst in dests.items():
                        rows = dst.shape[0]
                        nc.vector.tensor_scalar_add(
                            dst[:, ts(c, CH)],
                            pts[pi][0:rows, :],
                            bqk_sb[0:rows, pi:pi + 1],
                        )

            # K_h2 lives at partitions 64:128 of qk2_sb; realign to base 0
            nc.sync.dma_start(out=kT2_sb, in_=qk2_sb[64:128, :])

            # ---- V transposes: [d', t-slice] -> [t, d'] + ones columns ----
            with tc.tile_pool(name="ptr", bufs=4, space="PSUM") as ptr:
                for t in range(ST):
                    tr1 = ptr.tile([128, 128], dt.bfloat16, tag="tr1",
                                   name="tr1")
                    nc.tensor.transpose(tr1, v01_sb[:, ts(t, 128)], ident)
                    nc.vector.tensor_copy(
                        vv01[t].rearrange("p (h x) -> p h x", h=2)[:, :, 0:HD],
                        tr1.rearrange("p (h x) -> p h x", h=2),
                    )
                    nc.gpsimd.memset(
                        vv01[t].rearrange("p (h x) -> p h x", h=2)[:, :,
                                                                   HD:HD + 1],
                        1.0)
                    tr2 = ptr.tile([128, HD], dt.bfloat16, tag="tr2",
                                   name="tr2")
                    nc.tensor.transpose(tr2, v2_sb[:, ts(t, 128)],
                                        ident[0:HD, 0:HD])
                    nc.vector.tensor_copy(vv2[t][:, 0:HD], tr2)
                    nc.gpsimd.memset(vv2[t][:, HD:HD + 1], 1.0)

            qk_heads = [
                (qk01_sb[0:HD, :], k01_sb[0:HD, :]),
                (qk01_sb[HD:128, :], k01_sb[HD:128, :]),
                (qk2_sb[0:HD, :], kT2_sb),
            ]
            vv_heads = [
                lambda t: vv01[t][:, 0:HD + 1],
                lambda t: vv01[t][:, HD + 1:2 * (HD + 1)],
                lambda t: vv2[t],
            ]

            # ---------------- attention (pipelined per t-tile) --------------
            with (
                tc.tile_pool(name="ps", bufs=2, space="PSUM") as psp,
                tc.tile_pool(name="po", bufs=1, space="PSUM") as pop,
                tc.tile_pool(name="exps", bufs=4) as epool,
                tc.tile_pool(name="smalls", bufs=4) as spool,
            ):
                for h in range(HPC):
                    qT_h, kT_h = qk_heads[h]
                    vv_h = vv_heads[h]
                    po = [pop.tile([HD + 1, CH], dt.float32, tag=f"po{c}",
                                   name=f"po{c}")
                          for c in range(NCH)]
                    for t in range(ST):
                        for half in range(2):
                            ps = psp.tile([128, 2 * CH], dt.float32, tag="ps",
                                          name="ps")
                            for j in range(2):
                                c = 2 * half + j
                                nc.tensor.matmul(
                                    ps[:, ts(j, CH)],
                                    kT_h[:, ts(t, 128)],
                                    qT_h[:, ts(c, CH)],
                                )
                            et = epool.tile([128, 2 * CH], dt.bfloat16,
                                            tag="exp", name="et")
                            nc.scalar.activation(et, ps, AF.Exp, scale=0.125)
                            for j in range(2):
                                c = 2 * half + j
                                nc.tensor.matmul(
                                    po[c],
                                    vv_h(t),
                                    et[:, ts(j, CH)],
                                    start=(t == 0),
                                    stop=(t == ST - 1),
                                )
                    # normalization per chunk
                    for c in range(NCH):
                        dtile = spool.tile([1, CH], dt.float32, tag="den",
                                           name="dtile")
                        nc.vector.tensor_copy(dtile, po[c][HD:HD + 1, :])
                        rtile = spool.tile([1, CH], dt.float32, tag="rec",
                                           name="rtile")
                        nc.vector.reciprocal_approx_fast(out=rtile, in_=dtile)
                        bcst = spool.tile([HD, CH], dt.float32, tag="bcast",
                                          name="bcst")
                        nc.gpsimd.partition_broadcast(bcst, rtile)
                        dst = onrm_sb[h][:, ts(c, CH)]
                        nc.vector.tensor_mul(dst, po[c][0:HD, :], bcst)
                        nc.vector.tensor_scalar_add(dst, dst,
                                                    bv_sb[:, h:h + 1])

            # ---------------- output projection ----------------
            with (
                tc.tile_pool(name="pout", bufs=2, space="PSUM") as poutp,
                tc.tile_pool(name="outs", bufs=2) as opool,
            ):
                for e in range(KT):
                    ot = opool.tile([128, S], dt.bfloat16, tag="ot", name="ot")
                    for c in range(NCH):
                        pout = poutp.tile([128, CH], dt.float32, tag="pout",
                                          name="pout")
                        for h in range(HPC):
                            nc.tensor.matmul(
                                pout,
                                wo_sb[:, h * DM + e * 128: h * DM + (e + 1) * 128],
                                onrm_sb[h][:, ts(c, CH)],
                                start=(h == 0),
                                stop=(h == HPC - 1),
                            )
                        nc.vector.tensor_copy(ot[:, ts(c, CH)], pout)
                    nc.sync.dma_start(out=out_pT[ts(e, 128), :], in_=ot)

        if loop_reps > 1:
            with tc.For_i(0, loop_reps, 1):
                body()
        else:
            body()

    nc.compile()
    return nc


def _shard_inputs(query, key, value, wq, bq, wk, bk, wv, bv, wo, bo):
    """Build the 8 per-core input maps."""
    f32 = np.float32
    in_maps = []
    for core in range(NCORES):
        b = core // 4
        h0 = (core % 4) * HPC
        cs = slice(h0 * HD, (h0 + HPC) * HD)
        wq_s = np.asarray(wq[:, cs], f32)
        wk_s = np.asarray(wk[:, cs], f32)
        wv_s = np.asarray(wv[:, cs], f32)
        pad = np.zeros((DM, HD), f32)
        wcat = np.concatenate(
            [wq_s[:, 0:128], wk_s[:, 0:128], wq_s[:, 128:192],
             wk_s[:, 128:192], wv_s[:, 0:128], wv_s[:, 128:192], pad], axis=1)
        bq_s, bk_s, bv_s = (np.asarray(x[cs], f32) for x in (bq, bk, bv))
        bcat = np.concatenate([bq_s[0:128], bk_s[0:128], bq_s[128:192],
                               bk_s[128:192], bv_s[0:128], bv_s[128:192],
                               np.zeros(64, f32)])
        in_maps.append({
            "xqT": np.ascontiguousarray(np.asarray(query, f32)[b].T).astype(BF16),
            "xkT": np.ascontiguousarray(np.asarray(key, f32)[b].T).astype(BF16),
            "xvT": np.ascontiguousarray(np.asarray(value, f32)[b].T).astype(BF16),
            "wcat": np.ascontiguousarray(wcat).astype(BF16),
            "wo": np.ascontiguousarray(np.asarray(wo, f32)[cs, :]).astype(BF16),
            "bcat": np.ascontiguousarray(bcat),
            "bv": np.ascontiguousarray(bv_s),
        })
    return in_maps


def kernel(query, key, value, wq, bq, wk, bk, wv, bv, wo, bo):
    global _compiled
    from concourse.bass_utils import run_bass_kernel_spmd

    if _compiled is None:
        _compiled = _build()
    nc = _compiled

    in_maps = _shard_inputs(query, key, value, wq, bq, wk, bk, wv, bv, wo, bo)
    res = run_bass_kernel_spmd(nc, in_maps, list(range(NCORES)))

    out = np.zeros((B, S, DM), dtype=np.float32)
    for core in range(NCORES):
        b = core // 4
        out[b] += res.results[core]["out_pT"].astype(np.float32).T
    out += np.asarray(bo, dtype=np.float32)[None, None, :]
    return out
